# revision 8
# baseline (speedup 1.0000x reference)
"""MultiHeadAttention TRN2 Bass kernel (v2, fp16 datapath).

Problem: S=2048, B=2, H=16, d_k=64, D=1024, fp32 interface.
  q = query @ Wq.T + bq ; k = key @ Wk.T + bk ; v = value @ Wv.T + bv
  score = einsum('qbhd,kbhd->qkbh', q, k) / 8 ; attn = softmax(score, axis=k)
  out = einsum('qkbh,kbhd->qbhd', attn, v) -> reshape -> @ Wo.T + bo

Sharding (8 cores): core c handles batch b = c//4 and heads [4*(c%4), 4*(c%4)+4).
Each core computes its partial output projection (tensor-parallel along the
head dim); the host sums the 4 partials per batch and adds the bias terms
(bv @ Wo.T + bo, the linear-foldable bias contributions).

v2 changes vs v1:
  * fp16 device datapath (host casts inputs/weights to fp16; DMA bytes halve;
    PE speed identical to f32r at 1 cycle/row, PSUM still accumulates fp32).
  * V head slots widened to 128 cols: [64 replicated ones | 64 value dims].
    The PV matmul's cost is free-dim bound, so the previously idle M columns
    now produce the softmax denominator pre-broadcast across 64 partitions
    for free -> no broadcast matmul, no ACT evacuation copies. Ones first:
    custom-DVE ops (reciprocal) ignore input partition offsets, so the
    denominator must land at partition 0 of the PV PSUM tile.
  * Softmax normalize runs entirely on DVE straight out of PSUM:
    reciprocal_approx_fast(pv[64:128]) then tensor_mul into fp16 AC.
  * Weight DMAs split per 128-row chunk and interleaved with the first
    projection's x tiles so the PE starts ~1.5us in instead of ~18us.
  * Slim lead-in (K/V/Q for token block 0 only); the rest of the projections
    drain as zip units through the attention kb loop.

Per-core device layout (host pre-transposes + casts, pure data layout):
  xqT/xkT/xvT : [D=1024, T=2048] fp16   input slices, feature-major
  wqT/wkT/wvT : [1024, 256] fp16        Wq[rows,:].T etc (lhsT tiles directly)
  woT         : [256, 1024] fp16        Wo[:, cols].T (rhs tiles directly)
  bqv/bkv     : [256] fp32              projection biases for q/k
  out         : [2048, 1024] fp16       partial output (token-major)

On-chip dataflow per core:
  QT,KT feature-major [256, 2048] fp16; V token-major [2048, 4*128] fp16 with
  [64 d | 64 ones] per head slot. scoresT[k,q] = K @ Q.T per head (head pairs
  co-run on disjoint PE partition halves) -> exp on ScalarE (scale=1/8) ->
  attnT fp16 in SBUF -> PV matmul V.T @ attnT = [128, q]: rows 0-63 the head
  output, rows 64-127 the replicated denominator; DVE fast-reciprocal +
  multiply normalizes into fp16 AC; AC @ Wo.T streams out per q block.
"""

import os

os.environ.setdefault("MYCRO_LOCAL_CACHE", "1")

import numpy as np

import concourse.bass as bass
import concourse.tile as tile
from concourse import bacc, bass_utils, mybir


def _install_ntff_hook():
    """Provide antenv.axon_hooks when the image lacks it, so trace=True can
    capture NTFF profiles through the axon tunnel. Degrades silently."""
    import contextlib
    import ctypes
    import sys

    if "antenv.axon_hooks" in sys.modules:
        return
    so_path = "/opt/axon/libaxon_pjrt.so"
    if not os.path.exists(so_path):
        return
    try:
        lib = ctypes.CDLL(so_path)
        if not hasattr(lib, "axon_start_nrt_profile"):
            return
        lib.axon_start_nrt_profile.argtypes = [
            ctypes.POINTER(ctypes.c_int64),
            ctypes.c_size_t,
        ]
        lib.axon_start_nrt_profile.restype = ctypes.c_int64
        lib.axon_stop_nrt_profile.argtypes = [ctypes.c_char_p]
        lib.axon_stop_nrt_profile.restype = ctypes.c_int64

        @contextlib.contextmanager
        def _hook(output_dir, device_ids):
            import jax

            jax.devices()
            if device_ids:
                ids = (ctypes.c_int64 * len(device_ids))(*device_ids)
                rc = lib.axon_start_nrt_profile(ids, len(device_ids))
            else:
                rc = lib.axon_start_nrt_profile(None, 0)
            if rc != 0:
                raise RuntimeError(f"axon_start_nrt_profile rc={rc}")
            try:
                yield
            finally:
                n = lib.axon_stop_nrt_profile(str(output_dir).encode())
                print(f"ntff profile: {n} file(s) -> {output_dir}")

        import types

        mod = types.ModuleType("antenv.axon_hooks")
        mod.get_axon_ntff_profile_hook = lambda: _hook
        mod.set_axon_ntff_profile_hook = lambda h: None
        sys.modules["antenv.axon_hooks"] = mod
    except Exception:
        pass


_install_ntff_hook()

F32 = mybir.dt.float32
FP16 = mybir.dt.float16
AF = mybir.ActivationFunctionType

S = 2048          # sequence length
B = 2             # batch
H = 16            # total heads
DK = 64           # head dim
D = 1024          # model dim
NCORES = 8
HL = H // (NCORES // B)   # heads per core = 4
HC = HL * DK              # head cols per core = 256
T = S                     # tokens per core (one batch element)
P = 128
QB = 512                  # q block (matmul free dim)
NKB = T // P              # 16 k blocks
NQB = T // QB             # 4 q blocks
VW = 2 * DK               # 128: [64 value dims | 64 ones] per head slot


def build_module():
    nc = bacc.Bacc("TRN2", target_bir_lowering=False, debug=False)

    xqT = nc.dram_tensor("xqT", [D, T], FP16, kind="ExternalInput").ap()
    xkT = nc.dram_tensor("xkT", [D, T], FP16, kind="ExternalInput").ap()
    xvT = nc.dram_tensor("xvT", [D, T], FP16, kind="ExternalInput").ap()
    wqT = nc.dram_tensor("wqT", [D, HC], FP16, kind="ExternalInput").ap()
    wkT = nc.dram_tensor("wkT", [D, HC], FP16, kind="ExternalInput").ap()
    wvT = nc.dram_tensor("wvT", [D, HC], FP16, kind="ExternalInput").ap()
    woT = nc.dram_tensor("woT", [HC, D], FP16, kind="ExternalInput").ap()
    bqv = nc.dram_tensor("bqv", [HC], F32, kind="ExternalInput").ap()
    bkv = nc.dram_tensor("bkv", [HC], F32, kind="ExternalInput").ap()
    out = nc.dram_tensor("out", [T, D], FP16, kind="ExternalOutput").ap()

    with tile.TileContext(nc) as tc:
        kernel_body(tc, xqT, xkT, xvT, wqT, wkT, wvT, woT, bqv, bkv, out)

    nc.compile()
    return nc


def kernel_body(tc, xqT, xkT, xvT, wqT, wkT, wvT, woT, bqv, bkv, out):
    nc = tc.nc
    NKC = D // P  # 8 contraction chunks for projections

    with (
        tc.tile_pool(name="consts", bufs=1) as consts,
        tc.tile_pool(name="xs", bufs=18) as xs,
        tc.tile_pool(name="persist", bufs=1) as persist,
        tc.tile_pool(name="attn", bufs=6) as attn_pool,
        tc.tile_pool(name="small", bufs=4) as small,
        tc.tile_pool(name="outs", bufs=4) as outs,
        tc.tile_pool(name="ps_mm", bufs=2, space="PSUM") as ps_mm,
        tc.tile_pool(name="ps_sc", bufs=2, space="PSUM") as ps_sc,
        tc.tile_pool(name="ps_pv", bufs=2, space="PSUM") as ps_pv,
    ):
        # ------------- constants (DMA-ordered: K weights chunk-wise first) ---
        wkT_r = wkT.rearrange("(kc p) m -> p kc m", p=P)
        wk_s = consts.tile([P, NKC, HC], FP16)
        for kc in range(NKC):
            nc.sync.dma_start(wk_s[:, kc], wkT_r[:, kc])
        bk_s = consts.tile([P, HC // P], F32)
        nc.sync.dma_start(bk_s, bkv.rearrange("(m p) -> p m", p=P))

        # ------------- persistent activations ----------------
        QT = [persist.tile([P, T], FP16, name=f"QT{m}") for m in range(2)]
        KT = [persist.tile([P, T], FP16, name=f"KT{m}") for m in range(2)]
        V = persist.tile([P, NKB, HL * VW], FP16, name="V")
        AC = [persist.tile([P, T], FP16, name=f"AC{c}") for c in range(2)]

        # whole-tile memset: evacs overwrite the d-cols, ones cols remain 1.0
        nc.vector.memset(V, 1.0)

        # ------------- projections ----------------
        _xid = [0]

        def x_tile(xT, kc, tb, tag):
            _xid[0] += 1
            t = xs.tile([P, QB], FP16, tag="x", name=f"{tag}_{_xid[0]}")
            nc.sync.dma_start(t, xT[kc * P : (kc + 1) * P, tb * QB : (tb + 1) * QB])
            return t

        def proj_qk_direct(xT, w_s, b_s, dst, tag, tbs, ms):
            # x tiles shared across the m chunks
            xts = {(kc, tb): x_tile(xT, kc, tb, tag) for tb in tbs for kc in range(NKC)}
            for m in ms:
                pss = {
                    tb: ps_mm.tile([P, QB], F32, tag="mm", name=f"pd_{tag}{m}{tb}")
                    for tb in tbs
                }
                for kc in range(NKC):
                    for tb in tbs:
                        nc.tensor.matmul(
                            pss[tb],
                            lhsT=w_s[:, kc, m * P : (m + 1) * P],
                            rhs=xts[kc, tb],
                            start=(kc == 0),
                            stop=(kc == NKC - 1),
                        )
                for tb in tbs:
                    nc.vector.tensor_scalar_add(
                        dst[m][:, tb * QB : (tb + 1) * QB], pss[tb], b_s[:, m : m + 1]
                    )

        def proj_v_direct(tbs):
            for tb in tbs:
                xts = [x_tile(xvT, kc, tb, "xv") for kc in range(NKC)]
                for i in range(QB // P):
                    t128 = tb * (QB // P) + i
                    ps = ps_mm.tile([P, HC], F32, tag="mm", name=f"pd_v{t128}")
                    for kc in range(NKC):
                        nc.tensor.matmul(
                            ps,
                            lhsT=xts[kc][:, i * P : (i + 1) * P],
                            rhs=wv_s[:, kc, :],
                            start=(kc == 0),
                            stop=(kc == NKC - 1),
                        )
                    nc.vector.tensor_copy(
                        V[:, t128].rearrange("p (h c) -> p h c", c=VW)[:, :, DK:],
                        ps.rearrange("p (h c) -> p h c", c=DK),
                    )

        def proj_qk_units(xT, w_s, b_s, dst, tag, jobs):
            # zip style: per (m, tb) job, kc-outer with fresh x tiles so x
            # residency stays small; each job = 1 psum + 8 MM units + evac
            units = []
            for m, tb in jobs:
                st = {}

                def mk_start(m=m, tb=tb, st=st):
                    st["ps"] = ps_mm.tile(
                        [P, QB], F32, tag="mm", name=f"pz_{tag}{m}{tb}"
                    )

                units.append(mk_start)
                for kc in range(NKC):

                    def mk_mm(m=m, tb=tb, kc=kc, st=st):
                        xt = x_tile(xT, kc, tb, tag)
                        nc.tensor.matmul(
                            st["ps"],
                            lhsT=w_s[:, kc, m * P : (m + 1) * P],
                            rhs=xt,
                            start=(kc == 0),
                            stop=(kc == NKC - 1),
                        )

                    units.append(mk_mm)

                def mk_evac(m=m, tb=tb, st=st):
                    nc.vector.tensor_scalar_add(
                        dst[m][:, tb * QB : (tb + 1) * QB],
                        st["ps"],
                        b_s[:, m : m + 1],
                    )

                units.append(mk_evac)
            return units

        def proj_v_units(tbs):
            units = []
            for tb in tbs:
                st = {}

                def mk_load(tb=tb, st=st):
                    st["x"] = [x_tile(xvT, kc, tb, "xv") for kc in range(NKC)]

                units.append(mk_load)
                for i in range(QB // P):

                    def mk_block(tb=tb, i=i, st=st):
                        t128 = tb * (QB // P) + i
                        ps = ps_mm.tile([P, HC], F32, tag="mm", name=f"pz_v{t128}")
                        for kc in range(NKC):
                            nc.tensor.matmul(
                                ps,
                                lhsT=st["x"][kc][:, i * P : (i + 1) * P],
                                rhs=wv_s[:, kc, :],
                                start=(kc == 0),
                                stop=(kc == NKC - 1),
                            )
                        nc.vector.tensor_copy(
                            V[:, t128].rearrange("p (h c) -> p h c", c=VW)[:, :, DK:],
                            ps.rearrange("p (h c) -> p h c", c=DK),
                        )

                    units.append(mk_block)
            return units

        # Minimal lead-in: only what qb0/hp0's first k blocks consume — K tb0
        # chunk m0 (scores kb 0-3), Q tb0 chunk m0 (qb0 queries), V tb0
        # (PV kb 0-3). Everything else drains through the attention kb loop
        # in consumption order, so the first exp lands ~12us in instead of
        # ~60us and ScalarE (the steady-state limiter) starts early.
        wv_s = consts.tile([P, NKC, HC], FP16)
        wq_s = consts.tile([P, NKC, HC], FP16)
        bq_s = consts.tile([P, HC // P], F32)
        proj_qk_direct(xkT, wk_s, bk_s, KT, "xk", (0,), (0,))
        wqT_r = wqT.rearrange("(kc p) m -> p kc m", p=P)
        for kc in range(NKC):
            nc.sync.dma_start(wq_s[:, kc], wqT_r[:, kc])
        nc.sync.dma_start(bq_s, bqv.rearrange("(m p) -> p m", p=P))
        wvT_r = wvT.rearrange("(kc p) m -> p kc m", p=P)
        for kc in range(NKC):
            nc.sync.dma_start(wv_s[:, kc], wvT_r[:, kc])
        proj_qk_direct(xqT, wq_s, bq_s, QT, "xq", (0,), (0,))
        proj_v_direct((0,))

        # wo is only needed by the output projection — DMA it after stage A
        wo_s = consts.tile([P, HC // P, D], FP16)
        nc.sync.dma_start(wo_s, woT.rearrange("(c p) n -> p c n", p=P))

        # remaining projections, ordered by when attention needs them:
        # hp0 consumes K m0/V of tb1-3 at kb4/8/12; hp1 needs the m1 chunks
        # (incl. tb0's) from kb16; Q tb_i by qb_i start.
        zip_units = (
            proj_qk_units(xkT, wk_s, bk_s, KT, "xk", [(0, 1)])
            + proj_v_units((1,))
            + proj_qk_units(xkT, wk_s, bk_s, KT, "xk", [(0, 2)])
            + proj_v_units((2,))
            + proj_qk_units(xkT, wk_s, bk_s, KT, "xk", [(0, 3)])
            + proj_v_units((3,))
            + proj_qk_units(xkT, wk_s, bk_s, KT, "xk", [(1, 0)])
            + proj_qk_units(xqT, wq_s, bq_s, QT, "xq", [(1, 0)])
            + proj_qk_units(xkT, wk_s, bk_s, KT, "xk", [(1, 1), (1, 2), (1, 3)])
            + proj_qk_units(xqT, wq_s, bq_s, QT, "xq", [(0, 1), (1, 1)])
            + proj_qk_units(xqT, wq_s, bq_s, QT, "xq", [(0, 2), (1, 2)])
            + proj_qk_units(xqT, wq_s, bq_s, QT, "xq", [(0, 3), (1, 3)])
        )
        zq = list(zip_units)[::-1]  # pop from end

        def drain(n):
            for _ in range(n):
                if zq:
                    zq.pop()()

        # ---------------- attention ----------------
        # Head pairs (2*hp, 2*hp+1) run their score matmuls concurrently on
        # disjoint PE row groups (K=64 each, base partitions 0 / 64).
        for qb in range(NQB):
            for hp in range(2):
                m = hp  # heads (2*hp, 2*hp+1) live in QT/KT chunk m
                h0, h1 = 2 * hp, 2 * hp + 1
                pv0 = ps_pv.tile([P, QB], F32, tag="pv", name=f"pv_{qb}_{h0}")
                pv1 = ps_pv.tile([P, QB], F32, tag="pv", name=f"pv_{qb}_{h1}")

                def emit_pv(kb, at, pv0=pv0, pv1=pv1, h0=h0, h1=h1):
                    nc.tensor.matmul(
                        pv0,
                        lhsT=V[:, kb, VW * h0 : VW * (h0 + 1)],
                        rhs=at[:, :QB],
                        start=(kb == 0),
                        stop=(kb == NKB - 1),
                    )
                    nc.tensor.matmul(
                        pv1,
                        lhsT=V[:, kb, VW * h1 : VW * (h1 + 1)],
                        rhs=at[:, QB:],
                        start=(kb == 0),
                        stop=(kb == NKB - 1),
                    )

                # Software-pipelined: PV for block kb issues after the score
                # pair for kb+1, giving the exp a full score-pair of slack.
                prev = None
                for kb in range(NKB):
                    sc = ps_sc.tile(
                        [P, 2 * QB], F32, tag="sc", name=f"sc_{qb}_{hp}_{kb}"
                    )
                    nc.tensor.matmul(
                        sc[:, :QB],
                        lhsT=KT[m][0:DK, kb * P : (kb + 1) * P],
                        rhs=QT[m][0:DK, qb * QB : (qb + 1) * QB],
                        start=True,
                        stop=True,
                    )
                    nc.tensor.matmul(
                        sc[:, QB:],
                        lhsT=KT[m][DK:P, kb * P : (kb + 1) * P],
                        rhs=QT[m][DK:P, qb * QB : (qb + 1) * QB],
                        start=True,
                        stop=True,
                    )
                    at = attn_pool.tile(
                        [P, 2 * QB], FP16, tag="at", name=f"at_{qb}_{hp}_{kb}"
                    )
                    nc.scalar.activation(at, sc, AF.Exp, scale=0.125)
                    if prev is not None:
                        emit_pv(*prev)
                    prev = (kb, at)
                    # qb0 drains faster: K/V for tb1-3 are consumed by its own
                    # later k blocks (needs ~4.1 units/kb through kb28).
                    drain(5 if qb == 0 else 4)
                emit_pv(*prev)

                for h, pv in ((h0, pv0), (h1, pv1)):
                    off = 64 * (h % 2)
                    # rows 0-63 of pv hold the softmax denominator already
                    # replicated across 64 partitions (ones cols of V), rows
                    # 64-127 the head values: one DVE fast-reciprocal + one
                    # multiply, straight from PSUM. (The custom-DVE reciprocal
                    # ignores input partition offsets — its input must sit at
                    # partition 0, hence the ones-first slot layout.)
                    rcp_bc = small.tile([DK, QB], F32, tag="rcp", name=f"rcp_{qb}_{h}")
                    nc.vector.reciprocal_approx_fast(rcp_bc, pv[:DK, :])
                    nc.vector.tensor_mul(
                        AC[m][off : off + DK, qb * QB : (qb + 1) * QB],
                        pv[DK:P, :],
                        rcp_bc,
                    )

            # ---------------- output projection for this q block ----------------
            # c-inner over both n halves reuses each AC lhsT twice.
            for i in range(QB // P):
                t128 = qb * (QB // P) + i
                pss = [
                    ps_mm.tile([P, 512], F32, tag="mm", name=f"ps_o{t128}{n}")
                    for n in range(2)
                ]
                for c in range(2):
                    for n in range(2):
                        nc.tensor.matmul(
                            pss[n],
                            lhsT=AC[c][:, t128 * P : (t128 + 1) * P],
                            rhs=wo_s[:, c, n * 512 : (n + 1) * 512],
                            start=(c == 0),
                            stop=(c == 1),
                        )
                for n in range(2):
                    ob = outs.tile([P, 512], FP16, tag="ob", name=f"ob_{t128}_{n}")
                    nc.vector.tensor_copy(ob, pss[n])
                    nc.sync.dma_start(
                        out[t128 * P : (t128 + 1) * P, n * 512 : (n + 1) * 512], ob
                    )

        drain(len(zip_units))


_module_cache = None


def get_module():
    global _module_cache
    if _module_cache is None:
        _module_cache = build_module()
    return _module_cache


def shard_inputs(query, key, value, Wq, bq, Wk, bk, Wv, bv, Wo, bo):
    """Build the 8 per-core input maps (host-side layout transforms only)."""
    f = np.float32
    h = np.float16
    xT = {}
    for b in range(B):
        xT["q", b] = np.ascontiguousarray(np.asarray(query, f)[:, b, :].T.astype(h))
        xT["k", b] = np.ascontiguousarray(np.asarray(key, f)[:, b, :].T.astype(h))
        xT["v", b] = np.ascontiguousarray(np.asarray(value, f)[:, b, :].T.astype(h))
    Wq, Wk, Wv, Wo = (np.asarray(w, f) for w in (Wq, Wk, Wv, Wo))
    bq, bk = np.asarray(bq, f), np.asarray(bk, f)
    in_maps = []
    for c in range(NCORES):
        b, hg = c // (NCORES // B), c % (NCORES // B)
        cols = slice(HC * hg, HC * (hg + 1))
        in_maps.append(
            {
                "xqT": xT["q", b],
                "xkT": xT["k", b],
                "xvT": xT["v", b],
                "wqT": np.ascontiguousarray(Wq[cols, :].T.astype(h)),
                "wkT": np.ascontiguousarray(Wk[cols, :].T.astype(h)),
                "wvT": np.ascontiguousarray(Wv[cols, :].T.astype(h)),
                "woT": np.ascontiguousarray(Wo[:, cols].T.astype(h)),
                "bqv": np.ascontiguousarray(bq[cols]),
                "bkv": np.ascontiguousarray(bk[cols]),
            }
        )
    return in_maps


def kernel(query, key, value, Wq, bq, Wk, bk, Wv, bv, Wo, bo, trace=False):
    nc = get_module()
    in_maps = shard_inputs(query, key, value, Wq, bq, Wk, bk, Wv, bv, Wo, bo)
    res = bass_utils.run_bass_kernel_spmd(
        nc, in_maps, core_ids=list(range(NCORES)), trace=trace
    )
    f = np.float32
    bias_term = np.asarray(bv, f) @ np.asarray(Wo, f).T + np.asarray(bo, f)
    output = np.empty((S, B, D), f)
    for b in range(B):
        acc = res.results[4 * b]["out"].astype(f)
        for c in range(4 * b + 1, 4 * b + 4):
            acc = acc + res.results[c]["out"].astype(f)
        output[:, b, :] = acc + bias_term
    if trace:
        kernel.last_results = res
    return output


# revision 14
# speedup vs baseline: 1.1729x; 1.1729x over previous
"""MultiHeadAttention TRN2 Bass kernel (v4, fp16 datapath).

Problem: S=2048, B=2, H=16, d_k=64, D=1024, fp32 interface.
  q = query @ Wq.T + bq ; k = key @ Wk.T + bk ; v = value @ Wv.T + bv
  score = einsum('qbhd,kbhd->qkbh', q, k) / 8 ; attn = softmax(score, axis=k)
  out = einsum('qkbh,kbhd->qbhd', attn, v) -> reshape -> @ Wo.T + bo

Sharding (8 cores): core c handles batch b = c//4 and heads [4*(c%4), 4*(c%4)+4).
Each core computes its partial output projection (tensor-parallel along the
head dim); the host sums the 4 partials per batch and adds the bias terms
(bv @ Wo.T + bo, the linear-foldable bias contributions).

Key techniques (see git-style history in the module docstrings of prior
versions):
  * fp16 device datapath (host casts inputs/weights; DMA bytes halve; PE
    speed identical to f32r at 1 cycle/row; PSUM accumulates fp32).
  * V head slots are 128 cols: [64 replicated ones | 64 value dims]. The PV
    matmul cost is free-dim bound, so the otherwise idle M columns produce
    the softmax denominator pre-broadcast across 64 partitions for free.
    Ones first: custom-DVE ops ignore input partition offsets, so the
    denominator must sit at partition 0 for the fast reciprocal.
  * Softmax normalize fully on DVE straight from PSUM: fast-reciprocal of
    pv[0:64] then one multiply into fp16 AC.
  * Score matmuls for a head pair use disjoint PE partition halves (K=64 at
    base 0 / 64) and co-run on the PE array.
  * DMA: weights ride the SP queue as whole-tensor transfers; x tiles ride
    the DVE queue (a parallel HWDGE issue stream — each dma_start costs
    ~0.65us of queue time, so a single queue serializes startup).
  * The kb loop emits PV(kb-1) before scores(kb) so the PE stream alternates
    deterministically; exp(kb) on ScalarE is the steady-state limiter.
  * Each qb's output projection drains through the next qb's kb loop (the
    drain queue) instead of bursting at the qb boundary.

Per-core device layout (host pre-transposes + casts, pure data layout):
  xqT/xkT/xvT : [D=1024, T=2048] fp16   input slices, feature-major
  wqT/wkT/wvT : [1024, 256] fp16        Wq[rows,:].T etc (lhsT tiles directly)
  woT         : [256, 1024] fp16        Wo[:, cols].T (rhs tiles directly)
  bqv/bkv     : [256] fp32              projection biases for q/k
  out         : [2048, 1024] fp16       partial output (token-major)
"""

import os

os.environ.setdefault("MYCRO_LOCAL_CACHE", "1")

import numpy as np

import concourse.bass as bass
import concourse.tile as tile
from concourse import bacc, bass_utils, mybir


def _install_ntff_hook():
    """Provide antenv.axon_hooks when the image lacks it, so trace=True can
    capture NTFF profiles through the axon tunnel. Degrades silently."""
    import contextlib
    import ctypes
    import sys

    if "antenv.axon_hooks" in sys.modules:
        return
    so_path = "/opt/axon/libaxon_pjrt.so"
    if not os.path.exists(so_path):
        return
    try:
        lib = ctypes.CDLL(so_path)
        if not hasattr(lib, "axon_start_nrt_profile"):
            return
        lib.axon_start_nrt_profile.argtypes = [
            ctypes.POINTER(ctypes.c_int64),
            ctypes.c_size_t,
        ]
        lib.axon_start_nrt_profile.restype = ctypes.c_int64
        lib.axon_stop_nrt_profile.argtypes = [ctypes.c_char_p]
        lib.axon_stop_nrt_profile.restype = ctypes.c_int64

        @contextlib.contextmanager
        def _hook(output_dir, device_ids):
            import jax

            jax.devices()
            if device_ids:
                ids = (ctypes.c_int64 * len(device_ids))(*device_ids)
                rc = lib.axon_start_nrt_profile(ids, len(device_ids))
            else:
                rc = lib.axon_start_nrt_profile(None, 0)
            if rc != 0:
                raise RuntimeError(f"axon_start_nrt_profile rc={rc}")
            try:
                yield
            finally:
                n = lib.axon_stop_nrt_profile(str(output_dir).encode())
                print(f"ntff profile: {n} file(s) -> {output_dir}")

        import types

        mod = types.ModuleType("antenv.axon_hooks")
        mod.get_axon_ntff_profile_hook = lambda: _hook
        mod.set_axon_ntff_profile_hook = lambda h: None
        sys.modules["antenv.axon_hooks"] = mod
    except Exception:
        pass


_install_ntff_hook()

F32 = mybir.dt.float32
FP16 = mybir.dt.float16
AF = mybir.ActivationFunctionType

S = 2048          # sequence length
B = 2             # batch
H = 16            # total heads
DK = 64           # head dim
D = 1024          # model dim
NCORES = 8
HL = H // (NCORES // B)   # heads per core = 4
HC = HL * DK              # head cols per core = 256
T = S                     # tokens per core (one batch element)
P = 128
QB = 512                  # q block (matmul free dim)
NKB = T // P              # 16 k blocks
NQB = T // QB             # 4 q blocks
VW = 2 * DK               # 128: [64 ones | 64 value dims] per head slot


def build_module():
    nc = bacc.Bacc("TRN2", target_bir_lowering=False, debug=False)

    xqT = nc.dram_tensor("xqT", [D, T], FP16, kind="ExternalInput").ap()
    xkT = nc.dram_tensor("xkT", [D, T], FP16, kind="ExternalInput").ap()
    xvT = nc.dram_tensor("xvT", [D, T], FP16, kind="ExternalInput").ap()
    wqT = nc.dram_tensor("wqT", [D, HC], FP16, kind="ExternalInput").ap()
    wkT = nc.dram_tensor("wkT", [D, HC], FP16, kind="ExternalInput").ap()
    wvT = nc.dram_tensor("wvT", [D, HC], FP16, kind="ExternalInput").ap()
    woT = nc.dram_tensor("woT", [HC, D], FP16, kind="ExternalInput").ap()
    bqv = nc.dram_tensor("bqv", [HC], F32, kind="ExternalInput").ap()
    bkv = nc.dram_tensor("bkv", [HC], F32, kind="ExternalInput").ap()
    out = nc.dram_tensor("out", [T, D], FP16, kind="ExternalOutput").ap()

    with tile.TileContext(nc) as tc:
        kernel_body(tc, xqT, xkT, xvT, wqT, wkT, wvT, woT, bqv, bkv, out)

    nc.compile()
    return nc


def kernel_body(tc, xqT, xkT, xvT, wqT, wkT, wvT, woT, bqv, bkv, out):
    nc = tc.nc
    NKC = D // P   # 8 contraction chunks for projections
    NXP = NKC // 2  # 4 kc-pair x tiles per (tensor, tb)

    with (
        tc.tile_pool(name="consts", bufs=1) as consts,
        tc.tile_pool(name="xs", bufs=14) as xs,
        tc.tile_pool(name="persist", bufs=1) as persist,
        tc.tile_pool(name="attn", bufs=6) as attn_pool,
        tc.tile_pool(name="small", bufs=4) as small,
        tc.tile_pool(name="outs", bufs=4) as outs,
        tc.tile_pool(name="ps_mm", bufs=2, space="PSUM") as ps_mm,
        tc.tile_pool(name="ps_sc", bufs=2, space="PSUM") as ps_sc,
        tc.tile_pool(name="ps_pv", bufs=2, space="PSUM") as ps_pv,
    ):
        # ------------- x tiles (DVE DMA queue — parallel to SP) -------------
        _xid = [0]

        def x_load(xT, tb, tag):
            """Load one token block's 8 kc chunks as 4 kc-pair tiles."""
            _xid[0] += 1
            ts = []
            for j in range(NXP):
                t = xs.tile([P, 2, QB], FP16, tag="x", name=f"{tag}_{_xid[0]}_{j}")
                nc.gpsimd.dma_start(
                    t,
                    xT.rearrange("(kc p) t -> p kc t", p=P)[
                        :, 2 * j : 2 * j + 2, tb * QB : (tb + 1) * QB
                    ],
                )
                ts.append(t)
            return ts

        def x_ap(xts, kc):
            return xts[kc // 2][:, kc % 2, :]

        # ------------- constants (SP DMA queue) -------------
        wk_s = consts.tile([P, NKC, HC], FP16)
        nc.sync.dma_start(wk_s, wkT.rearrange("(kc p) m -> p kc m", p=P))
        bk_s = consts.tile([P, HC // P], F32)
        nc.sync.dma_start(bk_s, bkv.rearrange("(m p) -> p m", p=P))

        # ------------- persistent activations -------------
        QT = [persist.tile([P, T], FP16, name=f"QT{m}") for m in range(2)]
        KT = [persist.tile([P, T], FP16, name=f"KT{m}") for m in range(2)]
        V = persist.tile([P, NKB, HL * VW], FP16, name="V")
        AC = [persist.tile([P, T], FP16, name=f"AC{c}") for c in range(2)]

        # whole-tile memset: evacs overwrite the value cols, ones cols stay 1.0
        nc.vector.memset(V, 1.0)

        # ------------- projection emitters -------------
        def proj_qk(xts, w_s, b_s, dst, tag, tb, m):
            ps = ps_mm.tile([P, QB], F32, tag="mm", name=f"p_{tag}{m}{tb}")
            for kc in range(NKC):
                nc.tensor.matmul(
                    ps,
                    lhsT=w_s[:, kc, m * P : (m + 1) * P],
                    rhs=x_ap(xts, kc),
                    start=(kc == 0),
                    stop=(kc == NKC - 1),
                )
            nc.vector.tensor_scalar_add(
                dst[m][:, tb * QB : (tb + 1) * QB], ps, b_s[:, m : m + 1]
            )

        def proj_v_block(xts, tb, i):
            t128 = tb * (QB // P) + i
            ps = ps_mm.tile([P, HC], F32, tag="mm", name=f"p_v{t128}")
            for kc in range(NKC):
                nc.tensor.matmul(
                    ps,
                    lhsT=x_ap(xts, kc)[:, i * P : (i + 1) * P],
                    rhs=wv_s[:, kc, :],
                    start=(kc == 0),
                    stop=(kc == NKC - 1),
                )
            nc.vector.tensor_copy(
                V[:, t128].rearrange("p (h c) -> p h c", c=VW)[:, :, DK:],
                ps.rearrange("p (h c) -> p h c", c=DK),
            )

        def qk_units(xT, w_s, b_s, dst, tag, tb, ms):
            """Zip units for one token block: shared x load + per-m jobs."""
            st = {}
            units = [lambda st=st, tb=tb: st.__setitem__("x", x_load(xT, tb, tag))]
            for m in ms:
                for kc in range(NKC):

                    def mk_mm(m=m, kc=kc, st=st, tb=tb):
                        if kc == 0:
                            st["ps", m] = ps_mm.tile(
                                [P, QB], F32, tag="mm", name=f"pz_{tag}{m}{tb}"
                            )
                        nc.tensor.matmul(
                            st["ps", m],
                            lhsT=w_s[:, kc, m * P : (m + 1) * P],
                            rhs=x_ap(st["x"], kc),
                            start=(kc == 0),
                            stop=(kc == NKC - 1),
                        )

                    units.append(mk_mm)

                def mk_evac(m=m, st=st, tb=tb):
                    nc.vector.tensor_scalar_add(
                        dst[m][:, tb * QB : (tb + 1) * QB],
                        st["ps", m],
                        b_s[:, m : m + 1],
                    )

                units.append(mk_evac)
            return units

        def v_units(tb):
            st = {}
            units = [lambda st=st, tb=tb: st.__setitem__("x", x_load(xvT, tb, "xv"))]
            for i in range(QB // P):
                units.append(lambda i=i, st=st, tb=tb: proj_v_block(st["x"], tb, i))
            return units

        # ------------- stage A: K/V/Q for token blocks 0-1 -------------
        # Covers attention kb 0..7 and q blocks 0..1; tb2-3 + the output
        # projections drain through the attention kb loop.
        xk01 = {tb: x_load(xkT, tb, "xk") for tb in (0, 1)}
        wv_s = consts.tile([P, NKC, HC], FP16)
        nc.sync.dma_start(wv_s, wvT.rearrange("(kc p) m -> p kc m", p=P))
        wq_s = consts.tile([P, NKC, HC], FP16)
        nc.sync.dma_start(wq_s, wqT.rearrange("(kc p) m -> p kc m", p=P))
        bq_s = consts.tile([P, HC // P], F32)
        nc.sync.dma_start(bq_s, bqv.rearrange("(m p) -> p m", p=P))
        for m in (0, 1):
            for tb in (0, 1):
                proj_qk(xk01[tb], wk_s, bk_s, KT, "xk", tb, m)
        xv01 = {tb: x_load(xvT, tb, "xv") for tb in (0, 1)}
        for tb in (0, 1):
            for i in range(QB // P):
                proj_v_block(xv01[tb], tb, i)
        xq01 = {tb: x_load(xqT, tb, "xq") for tb in (0, 1)}
        wo_s = consts.tile([P, HC // P, D], FP16)
        nc.sync.dma_start(wo_s, woT.rearrange("(c p) n -> p c n", p=P))
        for m in (0, 1):
            for tb in (0, 1):
                proj_qk(xq01[tb], wq_s, bq_s, QT, "xq", tb, m)

        # remaining projections, ordered by when attention needs them:
        # tb2 by kb8, tb3 by kb12 (m0/hp0), m1 chunks by kb16+ (hp1),
        # Q tb2/tb3 by qb2/qb3.
        zip_units = (
            qk_units(xkT, wk_s, bk_s, KT, "xk", 2, (0, 1))
            + v_units(2)
            + qk_units(xkT, wk_s, bk_s, KT, "xk", 3, (0, 1))
            + v_units(3)
            + qk_units(xqT, wq_s, bq_s, QT, "xq", 2, (0, 1))
            + qk_units(xqT, wq_s, bq_s, QT, "xq", 3, (0, 1))
        )
        zq = list(zip_units)[::-1]  # pop from end

        def drain(n):
            for _ in range(n):
                if zq:
                    zq.pop()()

        def oproj_units(qb):
            """Output projection for one q block as drainable units."""
            units = []
            for i in range(QB // P):
                t128 = qb * (QB // P) + i
                st = {}

                def mk_mms(t128=t128, st=st):
                    pss = [
                        ps_mm.tile([P, 512], F32, tag="mm", name=f"ps_o{t128}{n}")
                        for n in range(2)
                    ]
                    for c in range(2):
                        for n in range(2):
                            nc.tensor.matmul(
                                pss[n],
                                lhsT=AC[c][:, t128 * P : (t128 + 1) * P],
                                rhs=wo_s[:, c, n * 512 : (n + 1) * 512],
                                start=(c == 0),
                                stop=(c == 1),
                            )
                    st["pss"] = pss

                def mk_out(t128=t128, st=st):
                    for n in range(2):
                        ob = outs.tile([P, 512], FP16, tag="ob", name=f"ob_{t128}_{n}")
                        nc.vector.tensor_copy(ob, st["pss"][n])
                        nc.sync.dma_start(
                            out[t128 * P : (t128 + 1) * P, n * 512 : (n + 1) * 512],
                            ob,
                        )

                units.extend([mk_mms, mk_out])
            return units

        # ---------------- attention ----------------
        # Head pairs (2*hp, 2*hp+1) run their score matmuls concurrently on
        # disjoint PE row groups (K=64 each, base partitions 0 / 64).
        for qb in range(NQB):
            for hp in range(2):
                m = hp  # heads (2*hp, 2*hp+1) live in QT/KT chunk m
                h0, h1 = 2 * hp, 2 * hp + 1
                pv0 = ps_pv.tile([P, QB], F32, tag="pv", name=f"pv_{qb}_{h0}")
                pv1 = ps_pv.tile([P, QB], F32, tag="pv", name=f"pv_{qb}_{h1}")

                def emit_pv(kb, at, pv0=pv0, pv1=pv1, h0=h0, h1=h1):
                    nc.tensor.matmul(
                        pv0,
                        lhsT=V[:, kb, VW * h0 : VW * (h0 + 1)],
                        rhs=at[:, :QB],
                        start=(kb == 0),
                        stop=(kb == NKB - 1),
                    )
                    nc.tensor.matmul(
                        pv1,
                        lhsT=V[:, kb, VW * h1 : VW * (h1 + 1)],
                        rhs=at[:, QB:],
                        start=(kb == 0),
                        stop=(kb == NKB - 1),
                    )

                # Software-pipelined: PV for block kb issues after the score
                # pair for kb+1, so the score pair runs on the PE while
                # exp(kb) executes on ScalarE.
                prev = None
                for kb in range(NKB):
                    sc = ps_sc.tile(
                        [P, 2 * QB], F32, tag="sc", name=f"sc_{qb}_{hp}_{kb}"
                    )
                    nc.tensor.matmul(
                        sc[:, :QB],
                        lhsT=KT[m][0:DK, kb * P : (kb + 1) * P],
                        rhs=QT[m][0:DK, qb * QB : (qb + 1) * QB],
                        start=True,
                        stop=True,
                    )
                    nc.tensor.matmul(
                        sc[:, QB:],
                        lhsT=KT[m][DK:P, kb * P : (kb + 1) * P],
                        rhs=QT[m][DK:P, qb * QB : (qb + 1) * QB],
                        start=True,
                        stop=True,
                    )
                    at = attn_pool.tile(
                        [P, 2 * QB], FP16, tag="at", name=f"at_{qb}_{hp}_{kb}"
                    )
                    nc.scalar.activation(at, sc, AF.Exp, scale=0.125)
                    if prev is not None:
                        emit_pv(*prev)
                    prev = (kb, at)
                    drain(5 if qb == 0 else 3)
                emit_pv(*prev)

                for h, pv in ((h0, pv0), (h1, pv1)):
                    off = 64 * (h % 2)
                    # rows 0-63 of pv: softmax denominator replicated across
                    # 64 partitions (ones cols of V); rows 64-127: the head
                    # values. One DVE fast-reciprocal + one multiply from
                    # PSUM. (The custom-DVE reciprocal ignores input
                    # partition offsets — its input must sit at partition 0.)
                    rcp_bc = small.tile([DK, QB], F32, tag="rcp", name=f"rcp_{qb}_{h}")
                    nc.vector.reciprocal_approx_fast(rcp_bc, pv[:DK, :])
                    nc.vector.tensor_mul(
                        AC[m][off : off + DK, qb * QB : (qb + 1) * QB],
                        pv[DK:P, :],
                        rcp_bc,
                    )

            # Output projection drains through the next qb's kb loop; the
            # last qb has no successor, so it emits directly.
            if qb < NQB - 1:
                zq.extend(oproj_units(qb)[::-1])  # popped next
            else:
                for u in oproj_units(qb):
                    u()

        drain(len(zip_units) + 64)


_module_cache = None


def get_module():
    global _module_cache
    if _module_cache is None:
        _module_cache = build_module()
    return _module_cache


def shard_inputs(query, key, value, Wq, bq, Wk, bk, Wv, bv, Wo, bo):
    """Build the 8 per-core input maps (host-side layout transforms only)."""
    f = np.float32
    h = np.float16
    xT = {}
    for b in range(B):
        xT["q", b] = np.ascontiguousarray(np.asarray(query, f)[:, b, :].T.astype(h))
        xT["k", b] = np.ascontiguousarray(np.asarray(key, f)[:, b, :].T.astype(h))
        xT["v", b] = np.ascontiguousarray(np.asarray(value, f)[:, b, :].T.astype(h))
    Wq, Wk, Wv, Wo = (np.asarray(w, f) for w in (Wq, Wk, Wv, Wo))
    bq, bk = np.asarray(bq, f), np.asarray(bk, f)
    in_maps = []
    for c in range(NCORES):
        b, hg = c // (NCORES // B), c % (NCORES // B)
        cols = slice(HC * hg, HC * (hg + 1))
        in_maps.append(
            {
                "xqT": xT["q", b],
                "xkT": xT["k", b],
                "xvT": xT["v", b],
                "wqT": np.ascontiguousarray(Wq[cols, :].T.astype(h)),
                "wkT": np.ascontiguousarray(Wk[cols, :].T.astype(h)),
                "wvT": np.ascontiguousarray(Wv[cols, :].T.astype(h)),
                "woT": np.ascontiguousarray(Wo[:, cols].T.astype(h)),
                "bqv": np.ascontiguousarray(bq[cols]),
                "bkv": np.ascontiguousarray(bk[cols]),
            }
        )
    return in_maps


def kernel(query, key, value, Wq, bq, Wk, bk, Wv, bv, Wo, bo, trace=False):
    nc = get_module()
    in_maps = shard_inputs(query, key, value, Wq, bq, Wk, bk, Wv, bv, Wo, bo)
    res = bass_utils.run_bass_kernel_spmd(
        nc, in_maps, core_ids=list(range(NCORES)), trace=trace
    )
    f = np.float32
    bias_term = np.asarray(bv, f) @ np.asarray(Wo, f).T + np.asarray(bo, f)
    output = np.empty((S, B, D), f)
    for b in range(B):
        acc = res.results[4 * b]["out"].astype(f)
        for c in range(4 * b + 1, 4 * b + 4):
            acc = acc + res.results[c]["out"].astype(f)
        output[:, b, :] = acc + bias_term
    if trace:
        kernel.last_results = res
    return output


# revision 22
# speedup vs baseline: 1.2286x; 1.0474x over previous
"""MultiHeadAttention TRN2 Bass kernel (v4, fp16 datapath).

Problem: S=2048, B=2, H=16, d_k=64, D=1024, fp32 interface.
  q = query @ Wq.T + bq ; k = key @ Wk.T + bk ; v = value @ Wv.T + bv
  score = einsum('qbhd,kbhd->qkbh', q, k) / 8 ; attn = softmax(score, axis=k)
  out = einsum('qkbh,kbhd->qbhd', attn, v) -> reshape -> @ Wo.T + bo

Sharding (8 cores): core c handles batch b = c//4 and heads [4*(c%4), 4*(c%4)+4).
Each core computes its partial output projection (tensor-parallel along the
head dim); the host sums the 4 partials per batch and adds the bias terms
(bv @ Wo.T + bo, the linear-foldable bias contributions).

Key techniques (see git-style history in the module docstrings of prior
versions):
  * fp16 device datapath (host casts inputs/weights; DMA bytes halve; PE
    speed identical to f32r at 1 cycle/row; PSUM accumulates fp32).
  * V head slots are 128 cols: [64 replicated ones | 64 value dims]. The PV
    matmul cost is free-dim bound, so the otherwise idle M columns produce
    the softmax denominator pre-broadcast across 64 partitions for free.
    Ones first: custom-DVE ops ignore input partition offsets, so the
    denominator must sit at partition 0 for the fast reciprocal.
  * Softmax normalize fully on DVE straight from PSUM: fast-reciprocal of
    pv[0:64] then one multiply into fp16 AC.
  * Score matmuls for a head pair use disjoint PE partition halves (K=64 at
    base 0 / 64) and co-run on the PE array.
  * DMA: weights ride the SP queue as whole-tensor transfers; x tiles ride
    the DVE queue (a parallel HWDGE issue stream — each dma_start costs
    ~0.65us of queue time, so a single queue serializes startup).
  * The kb loop emits PV(kb-1) before scores(kb) so the PE stream alternates
    deterministically; exp(kb) on ScalarE is the steady-state limiter.
  * Each qb's output projection drains through the next qb's kb loop (the
    drain queue) instead of bursting at the qb boundary.

Per-core device layout (host pre-transposes + casts, pure data layout):
  xqT/xkT/xvT : [D=1024, T=2048] fp16   input slices, feature-major
  wqT/wkT/wvT : [1024, 256] fp16        Wq[rows,:].T etc (lhsT tiles directly)
  woT         : [256, 1024] fp16        Wo[:, cols].T (rhs tiles directly)
  bqv/bkv     : [256] fp32              projection biases for q/k
  out         : [2048, 1024] fp16       partial output (token-major)
"""

import os

os.environ.setdefault("MYCRO_LOCAL_CACHE", "1")

import numpy as np

import concourse.bass as bass
import concourse.tile as tile
from concourse import bacc, bass_utils, mybir


def _install_ntff_hook():
    """Provide antenv.axon_hooks when the image lacks it, so trace=True can
    capture NTFF profiles through the axon tunnel. Degrades silently."""
    import contextlib
    import ctypes
    import sys

    if "antenv.axon_hooks" in sys.modules:
        return
    so_path = "/opt/axon/libaxon_pjrt.so"
    if not os.path.exists(so_path):
        return
    try:
        lib = ctypes.CDLL(so_path)
        if not hasattr(lib, "axon_start_nrt_profile"):
            return
        lib.axon_start_nrt_profile.argtypes = [
            ctypes.POINTER(ctypes.c_int64),
            ctypes.c_size_t,
        ]
        lib.axon_start_nrt_profile.restype = ctypes.c_int64
        lib.axon_stop_nrt_profile.argtypes = [ctypes.c_char_p]
        lib.axon_stop_nrt_profile.restype = ctypes.c_int64

        @contextlib.contextmanager
        def _hook(output_dir, device_ids):
            import jax

            jax.devices()
            if device_ids:
                ids = (ctypes.c_int64 * len(device_ids))(*device_ids)
                rc = lib.axon_start_nrt_profile(ids, len(device_ids))
            else:
                rc = lib.axon_start_nrt_profile(None, 0)
            if rc != 0:
                raise RuntimeError(f"axon_start_nrt_profile rc={rc}")
            try:
                yield
            finally:
                n = lib.axon_stop_nrt_profile(str(output_dir).encode())
                print(f"ntff profile: {n} file(s) -> {output_dir}")

        import types

        mod = types.ModuleType("antenv.axon_hooks")
        mod.get_axon_ntff_profile_hook = lambda: _hook
        mod.set_axon_ntff_profile_hook = lambda h: None
        sys.modules["antenv.axon_hooks"] = mod
    except Exception:
        pass


_install_ntff_hook()

F32 = mybir.dt.float32
FP16 = mybir.dt.float16
AF = mybir.ActivationFunctionType

S = 2048          # sequence length
B = 2             # batch
H = 16            # total heads
DK = 64           # head dim
D = 1024          # model dim
NCORES = 8
HL = H // (NCORES // B)   # heads per core = 4
HC = HL * DK              # head cols per core = 256
T = S                     # tokens per core (one batch element)
P = 128
QB = 512                  # q block (matmul free dim)
NKB = T // P              # 16 k blocks
NQB = T // QB             # 4 q blocks
VW = 2 * DK               # 128: [64 ones | 64 value dims] per head slot


def build_module():
    nc = bacc.Bacc("TRN2", target_bir_lowering=False, debug=False)

    NKC = D // P
    # weights arrive pre-arranged so each partition's data is one contiguous
    # 4KB run (128 big DMA descriptors instead of 1024 x 512B)
    xqT = nc.dram_tensor("xqT", [D, T], FP16, kind="ExternalInput").ap()
    xkT = nc.dram_tensor("xkT", [D, T], FP16, kind="ExternalInput").ap()
    xvT = nc.dram_tensor("xvT", [D, T], FP16, kind="ExternalInput").ap()
    wqT = nc.dram_tensor("wqT", [P, NKC, HC], FP16, kind="ExternalInput").ap()
    wkT = nc.dram_tensor("wkT", [P, NKC, HC], FP16, kind="ExternalInput").ap()
    wvT = nc.dram_tensor("wvT", [P, NKC, HC], FP16, kind="ExternalInput").ap()
    woT = nc.dram_tensor("woT", [P, HC // P, D], FP16, kind="ExternalInput").ap()
    bqv = nc.dram_tensor("bqv", [P, HC // P], F32, kind="ExternalInput").ap()
    bkv = nc.dram_tensor("bkv", [P, HC // P], F32, kind="ExternalInput").ap()
    out = nc.dram_tensor("out", [T, D], FP16, kind="ExternalOutput").ap()

    with tile.TileContext(nc) as tc:
        kernel_body(tc, xqT, xkT, xvT, wqT, wkT, wvT, woT, bqv, bkv, out)

    nc.compile()
    return nc


def kernel_body(tc, xqT, xkT, xvT, wqT, wkT, wvT, woT, bqv, bkv, out):
    nc = tc.nc
    NKC = D // P   # 8 contraction chunks for projections
    NXP = NKC // 2  # 4 kc-pair x tiles per (tensor, tb)

    with (
        tc.tile_pool(name="consts", bufs=1) as consts,
        tc.tile_pool(name="xs", bufs=14) as xs,
        tc.tile_pool(name="persist", bufs=1) as persist,
        tc.tile_pool(name="attn", bufs=6) as attn_pool,
        tc.tile_pool(name="small", bufs=4) as small,
        tc.tile_pool(name="outs", bufs=4) as outs,
        tc.tile_pool(name="ps_mm", bufs=2, space="PSUM") as ps_mm,
        tc.tile_pool(name="ps_sc", bufs=2, space="PSUM") as ps_sc,
        tc.tile_pool(name="ps_pv", bufs=2, space="PSUM") as ps_pv,
    ):
        # ------------- x tiles (DVE DMA queue — parallel to SP) -------------
        _xid = [0]

        def x_load(xT, tb, tag):
            """Load one token block's 8 kc chunks as 4 kc-pair tiles."""
            _xid[0] += 1
            ts = []
            for j in range(NXP):
                t = xs.tile([P, 2, QB], FP16, tag="x", name=f"{tag}_{_xid[0]}_{j}")
                nc.gpsimd.dma_start(
                    t,
                    xT.rearrange("(kc p) t -> p kc t", p=P)[
                        :, 2 * j : 2 * j + 2, tb * QB : (tb + 1) * QB
                    ],
                )
                ts.append(t)
            return ts

        def x_ap(xts, kc):
            return xts[kc // 2][:, kc % 2, :]

        # ------------- constants (SP DMA queue) -------------
        wk_s = consts.tile([P, NKC, HC], FP16)
        nc.sync.dma_start(wk_s, wkT)
        bk_s = consts.tile([P, HC // P], F32)
        nc.sync.dma_start(bk_s, bkv)

        # ------------- persistent activations -------------
        QT = [persist.tile([P, T], FP16, name=f"QT{m}") for m in range(2)]
        KT = [persist.tile([P, T], FP16, name=f"KT{m}") for m in range(2)]
        V = persist.tile([P, NKB, HL * VW], FP16, name="V")
        AC = [persist.tile([P, T], FP16, name=f"AC{c}") for c in range(2)]

        # whole-tile memset: evacs overwrite the value cols, ones cols stay 1.0
        nc.vector.memset(V, 1.0)

        # ------------- projection emitters -------------
        def proj_qk(xts, w_s, b_s, dst, tag, tb, m):
            ps = ps_mm.tile([P, QB], F32, tag="mm", name=f"p_{tag}{m}{tb}")
            for kc in range(NKC):
                nc.tensor.matmul(
                    ps,
                    lhsT=w_s[:, kc, m * P : (m + 1) * P],
                    rhs=x_ap(xts, kc),
                    start=(kc == 0),
                    stop=(kc == NKC - 1),
                )
            nc.vector.tensor_scalar_add(
                dst[m][:, tb * QB : (tb + 1) * QB], ps, b_s[:, m : m + 1]
            )

        def proj_v_block(xts, tb, i):
            t128 = tb * (QB // P) + i
            ps = ps_mm.tile([P, HC], F32, tag="mm", name=f"p_v{t128}")
            for kc in range(NKC):
                nc.tensor.matmul(
                    ps,
                    lhsT=x_ap(xts, kc)[:, i * P : (i + 1) * P],
                    rhs=wv_s[:, kc, :],
                    start=(kc == 0),
                    stop=(kc == NKC - 1),
                )
            nc.vector.tensor_copy(
                V[:, t128].rearrange("p (h c) -> p h c", c=VW)[:, :, DK:],
                ps.rearrange("p (h c) -> p h c", c=DK),
            )

        def qk_units(xT, w_s, b_s, dst, tag, tb, ms):
            """Zip units for one token block: shared x load + per-m jobs."""
            st = {}
            units = [lambda st=st, tb=tb: st.__setitem__("x", x_load(xT, tb, tag))]
            for m in ms:
                for kc in range(NKC):

                    def mk_mm(m=m, kc=kc, st=st, tb=tb):
                        if kc == 0:
                            st["ps", m] = ps_mm.tile(
                                [P, QB], F32, tag="mm", name=f"pz_{tag}{m}{tb}"
                            )
                        nc.tensor.matmul(
                            st["ps", m],
                            lhsT=w_s[:, kc, m * P : (m + 1) * P],
                            rhs=x_ap(st["x"], kc),
                            start=(kc == 0),
                            stop=(kc == NKC - 1),
                        )

                    units.append(mk_mm)

                def mk_evac(m=m, st=st, tb=tb):
                    nc.vector.tensor_scalar_add(
                        dst[m][:, tb * QB : (tb + 1) * QB],
                        st["ps", m],
                        b_s[:, m : m + 1],
                    )

                units.append(mk_evac)
            return units

        def v_units(tb):
            st = {}
            units = [lambda st=st, tb=tb: st.__setitem__("x", x_load(xvT, tb, "xv"))]
            for i in range(QB // P):
                units.append(lambda i=i, st=st, tb=tb: proj_v_block(st["x"], tb, i))
            return units

        # ------------- stage A: K/V/Q for token blocks 0-1 -------------
        # Covers attention kb 0..7 and q blocks 0..1; tb2-3 + the output
        # projections drain through the attention kb loop.
        xk01 = {tb: x_load(xkT, tb, "xk") for tb in (0, 1)}
        wv_s = consts.tile([P, NKC, HC], FP16)
        nc.sync.dma_start(wv_s, wvT)
        wq_s = consts.tile([P, NKC, HC], FP16)
        nc.sync.dma_start(wq_s, wqT)
        bq_s = consts.tile([P, HC // P], F32)
        nc.sync.dma_start(bq_s, bqv)
        for m in (0, 1):
            for tb in (0, 1):
                proj_qk(xk01[tb], wk_s, bk_s, KT, "xk", tb, m)
        xv01 = {tb: x_load(xvT, tb, "xv") for tb in (0, 1)}
        for tb in (0, 1):
            for i in range(QB // P):
                proj_v_block(xv01[tb], tb, i)
        xq01 = {tb: x_load(xqT, tb, "xq") for tb in (0, 1)}
        wo_s = consts.tile([P, HC // P, D], FP16)
        nc.sync.dma_start(wo_s, woT)
        for m in (0, 1):
            for tb in (0, 1):
                proj_qk(xq01[tb], wq_s, bq_s, QT, "xq", tb, m)

        # remaining projections, ordered by when attention needs them:
        # tb2 by kb8, tb3 by kb12 (m0/hp0), m1 chunks by kb16+ (hp1),
        # Q tb2/tb3 by qb2/qb3.
        zip_units = (
            qk_units(xkT, wk_s, bk_s, KT, "xk", 2, (0, 1))
            + v_units(2)
            + qk_units(xkT, wk_s, bk_s, KT, "xk", 3, (0, 1))
            + v_units(3)
            + qk_units(xqT, wq_s, bq_s, QT, "xq", 2, (0, 1))
            + qk_units(xqT, wq_s, bq_s, QT, "xq", 3, (0, 1))
        )
        zq = list(zip_units)[::-1]  # pop from end

        def drain(n):
            for _ in range(n):
                if zq:
                    zq.pop()()

        def oproj_units(qb):
            """Output projection for one q block as drainable units."""
            units = []
            for i in range(QB // P):
                t128 = qb * (QB // P) + i
                st = {}

                def mk_mm(t128=t128, st=st, n=0):
                    ps = ps_mm.tile([P, 512], F32, tag="mm", name=f"ps_o{t128}{n}")
                    for c in range(2):
                        nc.tensor.matmul(
                            ps,
                            lhsT=AC[c][:, t128 * P : (t128 + 1) * P],
                            rhs=wo_s[:, c, n * 512 : (n + 1) * 512],
                            start=(c == 0),
                            stop=(c == 1),
                        )
                    st[n] = ps

                def mk_out(t128=t128, st=st, n=0):
                    ob = outs.tile([P, 512], FP16, tag="ob", name=f"ob_{t128}_{n}")
                    nc.vector.tensor_copy(ob, st[n])
                    nc.sync.dma_start(
                        out[t128 * P : (t128 + 1) * P, n * 512 : (n + 1) * 512], ob
                    )

                for n in range(2):
                    units.append(lambda f=mk_mm, n=n: f(n=n))
                    units.append(lambda f=mk_out, n=n: f(n=n))
            return units

        # ---------------- attention ----------------
        # Head pairs (2*hp, 2*hp+1) run their score matmuls concurrently on
        # disjoint PE row groups (K=64 each, base partitions 0 / 64).
        for qb in range(NQB):
            for hp in range(2):
                m = hp  # heads (2*hp, 2*hp+1) live in QT/KT chunk m
                h0, h1 = 2 * hp, 2 * hp + 1
                pv0 = ps_pv.tile([P, QB], F32, tag="pv", name=f"pv_{qb}_{h0}")
                pv1 = ps_pv.tile([P, QB], F32, tag="pv", name=f"pv_{qb}_{h1}")

                def emit_pv(kb, at, pv0=pv0, pv1=pv1, h0=h0, h1=h1):
                    nc.tensor.matmul(
                        pv0,
                        lhsT=V[:, kb, VW * h0 : VW * (h0 + 1)],
                        rhs=at[:, :QB],
                        start=(kb == 0),
                        stop=(kb == NKB - 1),
                    )
                    nc.tensor.matmul(
                        pv1,
                        lhsT=V[:, kb, VW * h1 : VW * (h1 + 1)],
                        rhs=at[:, QB:],
                        start=(kb == 0),
                        stop=(kb == NKB - 1),
                    )

                # Software-pipelined with a 2-kb lag: PV for block kb issues
                # after the score pair for kb+2, so when the PE reaches the
                # pv pair, exp(kb) has long completed — the PE stream never
                # stalls on a just-finished exp, and ScalarE stays the sole
                # steady-state limiter.
                pend = []
                for kb in range(NKB):
                    sc = ps_sc.tile(
                        [P, 2 * QB], F32, tag="sc", name=f"sc_{qb}_{hp}_{kb}"
                    )
                    nc.tensor.matmul(
                        sc[:, :QB],
                        lhsT=KT[m][0:DK, kb * P : (kb + 1) * P],
                        rhs=QT[m][0:DK, qb * QB : (qb + 1) * QB],
                        start=True,
                        stop=True,
                    )
                    nc.tensor.matmul(
                        sc[:, QB:],
                        lhsT=KT[m][DK:P, kb * P : (kb + 1) * P],
                        rhs=QT[m][DK:P, qb * QB : (qb + 1) * QB],
                        start=True,
                        stop=True,
                    )
                    at = attn_pool.tile(
                        [P, 2 * QB], FP16, tag="at", name=f"at_{qb}_{hp}_{kb}"
                    )
                    nc.scalar.activation(at, sc, AF.Exp, scale=0.125)
                    pend.append((kb, at))
                    if len(pend) > 2:
                        emit_pv(*pend.pop(0))
                    drain(5 if qb == 0 else 2)
                for p in pend:
                    emit_pv(*p)

                for h, pv in ((h0, pv0), (h1, pv1)):
                    off = 64 * (h % 2)
                    # rows 0-63 of pv: softmax denominator replicated across
                    # 64 partitions (ones cols of V); rows 64-127: the head
                    # values. One DVE fast-reciprocal + one multiply from
                    # PSUM. (The custom-DVE reciprocal ignores input
                    # partition offsets — its input must sit at partition 0.)
                    rcp_bc = small.tile([DK, QB], F32, tag="rcp", name=f"rcp_{qb}_{h}")
                    nc.vector.reciprocal_approx_fast(rcp_bc, pv[:DK, :])
                    nc.vector.tensor_mul(
                        AC[m][off : off + DK, qb * QB : (qb + 1) * QB],
                        pv[DK:P, :],
                        rcp_bc,
                    )

            # Output projection drains through the next qb's kb loop; the
            # last qb has no successor, so it emits directly.
            if qb < NQB - 1:
                zq.extend(oproj_units(qb)[::-1])  # popped next
            else:
                for u in oproj_units(qb):
                    u()

        drain(len(zip_units) + 64)


_module_cache = None


def get_module():
    global _module_cache
    if _module_cache is None:
        _module_cache = build_module()
    return _module_cache


def shard_inputs(query, key, value, Wq, bq, Wk, bk, Wv, bv, Wo, bo):
    """Build the 8 per-core input maps (host-side layout transforms only)."""
    f = np.float32
    h = np.float16
    xT = {}
    for b in range(B):
        xT["q", b] = np.ascontiguousarray(np.asarray(query, f)[:, b, :].T.astype(h))
        xT["k", b] = np.ascontiguousarray(np.asarray(key, f)[:, b, :].T.astype(h))
        xT["v", b] = np.ascontiguousarray(np.asarray(value, f)[:, b, :].T.astype(h))
    Wq, Wk, Wv, Wo = (np.asarray(w, f) for w in (Wq, Wk, Wv, Wo))
    bq, bk = np.asarray(bq, f), np.asarray(bk, f)

    def w_arr(WT):
        # [D, HC] -> [P, NKC, HC]: partition-contiguous for big DMA descriptors
        kc = WT.shape[0] // P
        return np.ascontiguousarray(WT.reshape(kc, P, -1).transpose(1, 0, 2).astype(h))

    def b_arr(bv_):
        return np.ascontiguousarray(bv_.reshape(-1, P).T)

    in_maps = []
    for c in range(NCORES):
        b, hg = c // (NCORES // B), c % (NCORES // B)
        cols = slice(HC * hg, HC * (hg + 1))
        in_maps.append(
            {
                "xqT": xT["q", b],
                "xkT": xT["k", b],
                "xvT": xT["v", b],
                "wqT": w_arr(Wq[cols, :].T),
                "wkT": w_arr(Wk[cols, :].T),
                "wvT": w_arr(Wv[cols, :].T),
                "woT": w_arr(Wo[:, cols].T),
                "bqv": b_arr(bq[cols]),
                "bkv": b_arr(bk[cols]),
            }
        )
    return in_maps


def kernel(query, key, value, Wq, bq, Wk, bk, Wv, bv, Wo, bo, trace=False):
    nc = get_module()
    in_maps = shard_inputs(query, key, value, Wq, bq, Wk, bk, Wv, bv, Wo, bo)
    res = bass_utils.run_bass_kernel_spmd(
        nc, in_maps, core_ids=list(range(NCORES)), trace=trace
    )
    f = np.float32
    bias_term = np.asarray(bv, f) @ np.asarray(Wo, f).T + np.asarray(bo, f)
    output = np.empty((S, B, D), f)
    for b in range(B):
        acc = res.results[4 * b]["out"].astype(f)
        for c in range(4 * b + 1, 4 * b + 4):
            acc = acc + res.results[c]["out"].astype(f)
        output[:, b, :] = acc + bias_term
    if trace:
        kernel.last_results = res
    return output
